# revision 25
# baseline (speedup 1.0000x reference)
"""BLIP3o DiT block on 8 Trainium2 NeuronCores.

Strategy: data-parallel over batch (32 batches -> 4 per core), zero collectives.
v5: bf16 weights/activations (same PE rate as fp32r here, half the DMA bytes
and SBUF), host pre-tiled weight layouts (contiguous 2-8KB DMA packet lines
instead of 512B -> ~21B/ns/queue), mod phase recomputed stemb-stationary
(free-dim 512 instead of 4, all 12 chunks at startup to fill the rms1/rope
latency) + PE transposes into column layout, eva at free-dim 512, attention
with kc-merged score PSUM + one exp per head and deep rings (sc3/av3/den2
banks; ps_proj released around each attention), n2 folded into wq2 on host
(rms2 writes normalized activations directly - avoids a bf16 single-scalar
tensor_scalar firmware slow path), single-pass 32-chunk down accumulation
with fused gated residual + store.

On-chip layout: activations transposed [feature, token]; fp32 residual
stream (hsT) and PSUM; bf16 matmul operands. GPSIMD never touches PSUM
(hardware restriction); binary DVE/GPSIMD ops keep both inputs
partition-aligned; at most one PSUM input per instruction.

Per-core dataflow (T = 4*256 = 1024 tokens):
  rms1 stats -> mod j=0..11 (stemb-stationary, bias via K=1 ones matmul)
  -> rms1 apply+modulate -> rope (in place) -> V1 -> Q1,K1 -> attn1
  (scoresT [128,2,S] PSUM, exp per head, AV + ones-matmul den, reciprocal
  -> mult) -> o1 gated resid -> rms2 stats -> eva (E=4096 contraction)
  -> k2,v2 -> rms2 apply (direct) -> q2 -> attn2 -> o2 resid
  -> rms3 -> gate/up (silu*up -> mlpT bf16) -> down + gated resid -> outT
"""
import os
import sys
import numpy as np

if "/root/pylocal" not in sys.path:
    sys.path.insert(0, "/root/pylocal")  # antenv.axon_hooks shim (NTFF tracing)
try:
    import antenv
    if "/root/pylocal/antenv" not in list(antenv.__path__):
        antenv.__path__.append("/root/pylocal/antenv")
except Exception:
    pass

import concourse.bass as bass
from concourse import bacc
import concourse.mybir as mybir
from concourse.tile import TileContext
from concourse.bass_utils import run_bass_kernel_spmd

F32 = mybir.dt.float32
BF = mybir.dt.bfloat16
AF = mybir.ActivationFunctionType
OP = mybir.AluOpType

B, S, L, H, NH, HD, I, E = 32, 256, 256, 1024, 16, 64, 4096, 4096
EPS = 1e-6
GRID = 16
NC_ = 8            # cores
BPC = B // NC_     # batches per core = 4
T = BPC * S        # tokens per core = 1024
HC = H // 128      # 8 feature chunks
EC = E // 128      # 32
IC = I // 128      # 32


def _rope_tables():
    q = H // 4
    inv = 1.0 / (10000.0 ** (np.arange(0, q, 2, dtype=np.float64) / q))  # [128]
    qd = 128
    pos_x = np.repeat(np.arange(GRID, dtype=np.float64), GRID)  # [S]
    pos_y = np.tile(np.arange(GRID, dtype=np.float64), GRID)
    fx = pos_x[:, None] * inv[None, :qd]   # [S, 128]
    fy = pos_y[:, None] * inv[None, :qd]
    t = lambda a: np.ascontiguousarray(
        np.tile(a.T.astype(np.float32), (1, BPC)))  # [128, S] -> [128, T]
    return t(np.cos(fx)), t(np.sin(fx)), t(np.cos(fy)), t(np.sin(fy))


def build_program():
    nc = bacc.Bacc()

    # ---------------- DRAM params ----------------
    d = {}
    def P(name, shape, dt, out=False):
        d[name] = nc.declare_dram_parameter(name, list(shape), dt, isOutput=out)
        return d[name]

    P("hsT4", [128, HC, T], F32)
    P("enc4", [128, EC, T], BF)
    P("stembT", [128, HC, BPC], F32)
    for w in ["wq1t", "wk1t", "wo1t", "wq2t", "wk2t", "wo2t"]:
        P(w, [HC, 128, HC, 128], BF)
    P("wv1t", [2, 128, HC, 512], BF)
    P("wv2t", [2, 128, HC, 512], BF)
    P("evat", [HC, 128, EC, 128], BF)
    P("adat", [12, 128, HC, 512], BF)
    P("adab", [1, 6 * H], BF)
    P("gatet", [IC, 128, HC, 128], BF)
    P("upt", [IC, 128, HC, 128], BF)
    P("downt", [HC, 128, IC, 128], BF)
    P("eva_bT", [128, HC], F32)
    for n in ["n1T", "n3T"]:
        P(n, [128, HC], F32)
    for tb in ["cxt", "sxt", "cyt", "syt"]:
        P(tb, [128, T], BF)
    P("ones_bf", [128, 128], BF)
    P("eye4", [4, 4], F32)
    P("epsc", [128, 1], F32)
    outT_d = P("outT", [128, HC, T], F32, out=True)

    tc_cm = TileContext(nc)
    tc = tc_cm.__enter__()

    open_pools = {}

    def pool(name, bufs=1, side="left"):
        p = tc.alloc_tile_pool(name=name, bufs=bufs, side=side)
        open_pools[name] = p
        return p

    def free(name):
        open_pools.pop(name).release()

    # long-lived pools
    wpool = pool("wstream", bufs=6)         # tag "w8": [128, 8, 128] bf16
    sml = pool("sml", bufs=1)
    const = pool("const", bufs=1)
    P_ = {"n": 0}

    def proj_alloc():
        P_["proj"] = tc.alloc_tile_pool(name=f"ps_proj{P_['n']}", bufs=2,
                                        space="PSUM")
        P_["n"] += 1

    def proj_release():
        P_["proj"].release()

    proj_alloc()

    # ---------------- constants ----------------
    ones_sb = const.tile([128, 128], BF)
    nc.sync.dma_start(ones_sb[:], d["ones_bf"][:])
    eye4_sb = const.tile([4, 4], F32)
    nc.sync.dma_start(eye4_sb[:], d["eye4"][:])
    eps_sb = const.tile([128, 1], F32)
    nc.sync.dma_start(eps_sb[:], d["epsc"][:])
    n_sb = {}
    for w in ["n1T", "n3T", "eva_bT"]:
        n_sb[w] = const.tile([128, HC], F32, name=w + "_sb")
        nc.sync.dma_start(n_sb[w][:], d[w][:])
    adab_sb = const.tile([1, 6 * H], BF)
    nc.sync.dma_start(adab_sb[:], d["adab"][:])

    # mod splits: sh_msa, sc_msa, g_msa, sh_mlp, sc_mlp, g_mlp
    modsp = [const.tile([128, HC, BPC], F32, name=f"modsp{i}") for i in range(6)]
    scale1 = const.tile([128, HC, BPC], F32)    # n1*(1+sc_msa)
    scale3 = const.tile([128, HC, BPC], F32)    # n3*(1+sc_mlp)

    # ---------------- residual stream ----------------
    p_hs = pool("p_hs")
    hsT = p_hs.tile([128, HC, T], F32)          # h, then h1, then h2 in place
    for c in range(HC):
        nc.sync.dma_start(hsT[:, c], d["hsT4"][:, c])

    # ---------------- rms helpers (stats / apply split) ----------------
    def rms_stats(name):
        """Returns (pool_name, ms tiles). mean-of-squares via ones-matmul."""
        ps_ms = tc.alloc_tile_pool(name="ps_ms_" + name, bufs=1, space="PSUM")
        open_pools["ps_ms_" + name] = ps_ms
        rt = pool("rsq_" + name, side="right")
        ms = [ps_ms.tile([128, 512], F32, name=f"ms_{name}_{t}")
              for t in range(2)]
        for c in range(HC):
            sq = rt.tile([128, T], BF, tag="sq", bufs=3, name=f"sq_{name}")
            nc.scalar.activation(sq[:], hsT[:, c], AF.Square)
            for t in range(2):
                nc.tensor.matmul(ms[t][:], ones_sb[:], sq[:, t * 512:(t + 1) * 512],
                                 start=(c == 0), stop=(c == HC - 1))
        free("rsq_" + name)
        return ms

    def rms_apply(ms, name, consumer=None, xn_dst=None):
        rtmp = pool("rtmp_" + name, side="right")
        for t in range(2):
            sroot = rtmp.tile([128, 512], F32, tag="sroot", bufs=2,
                              name=f"sroot_{name}")
            nc.scalar.activation(sroot[:], ms[t][:], AF.Sqrt,
                                 bias=eps_sb[:, 0:1], scale=1.0 / H)
            invn = rtmp.tile([128, 512], F32, tag="invn", bufs=2,
                             name=f"invn_{name}")
            nc.vector.reciprocal_approx_fast(invn[:], sroot[:])
            for c in range(HC):
                if xn_dst is not None:
                    xn = xn_dst(c, t)
                else:
                    xn = rtmp.tile([128, 512], BF, tag="xn", bufs=6,
                                   name=f"xn_{name}")
                eng = nc.vector if (c + t) % 2 == 0 else nc.gpsimd
                eng.tensor_tensor(xn[:], hsT[:, c, t * 512:(t + 1) * 512],
                                  invn[:], OP.mult)
                if consumer is not None:
                    consumer(c, t, xn)
        free("rtmp_" + name)
        free("ps_ms_" + name)

    # ---------------- mod: stemb-stationary + PE transpose -------------------
    stembT_sb = const.tile([128, HC, BPC], F32)
    nc.sync.dma_start(stembT_sb[:], d["stembT"][:])
    stemb = const.tile([128, HC, BPC], BF)
    nc.scalar.activation(stemb[:], stembT_sb[:], AF.Silu)

    ada_pool = pool("p_ada", bufs=6)
    modtmp = pool("p_modtmp")

    def mod_chunk(j, ps_t):
        """Computes modsp[j//2][:, (j%2)*4:(j%2)*4+4, :] (4 o-chunks)."""
        with nc.named_scope(f"mod{j}"):
            adt = ada_pool.tile([128, HC, 512], BF, tag="ada", name="ada_t")
            nc.sync.dma_start(adt[:], d["adat"][j])
            mp = P_["proj"].tile([128, 512], F32, tag="proj", name="mod_ps")
            for c in range(HC):
                nc.tensor.matmul(mp[0:4, :], stemb[:, c], adt[:, c],
                                 start=(c == 0), stop=False)
            # bias: psum[b, n] += 1 * adab[n]
            nc.tensor.matmul(mp[0:4, :], ones_sb[0:1, 0:4],
                             adab_sb[:, j * 512:(j + 1) * 512],
                             start=False, stop=True)
            m4 = modtmp.tile([4, 512], F32, tag="m4", bufs=2, name="mod4_sb")
            nc.scalar.copy(m4[:], mp[0:4, :])
            sp, oo = j // 2, (j % 2) * 4
            for k in range(4):
                tp = ps_t.tile([128, 4], F32, tag="tp", name="mod_tp")
                nc.tensor.transpose(tp[:], m4[0:4, k * 128:(k + 1) * 128],
                                    eye4_sb[0:4, 0:4])
                nc.scalar.copy(modsp[sp][:, oo + k, :], tp[:])

    # phase 0: rms1 stats overlap with mod j=0..3
    ps_t = tc.alloc_tile_pool(name="ps_t", bufs=2, space="PSUM")
    ms1 = rms_stats("r1")
    for j in range(12):
        mod_chunk(j, ps_t)
    with nc.named_scope("scales1"):
        for c in range(HC):
            nc.vector.tensor_scalar(scale1[:, c], modsp[1][:, c], 1.0,
                                    n_sb["n1T"][:, c:c + 1], OP.add, OP.mult)

    # ---------------- rms1 apply + modulate -> x1T; rope in place ------------
    p_rope = pool("p_rope")
    rope_t = {}
    for tb in ["cxt", "sxt", "cyt", "syt"]:
        rope_t[tb] = p_rope.tile([128, T], BF, name=tb + "_sb")
        nc.sync.dma_start(rope_t[tb][:], d[tb][:])

    p_x1 = pool("p_x1", side="right")
    x1T = p_x1.tile([128, HC, T], BF)

    def rms1_consumer(c, t, xn):
        for b2 in range(2):
            b = 2 * t + b2
            eng = nc.gpsimd if b2 == 0 else nc.vector
            eng.tensor_scalar(x1T[:, c, b * S:(b + 1) * S],
                              xn[:, b2 * S:(b2 + 1) * S],
                              scale1[:, c, b:b + 1],
                              modsp[0][:, c, b:b + 1],
                              OP.mult, OP.add)

    with nc.named_scope("rms1"):
        rms_apply(ms1, "r1", consumer=rms1_consumer)

    with nc.named_scope("rope"):
        rp = pool("p_ropetmp")
        for (i0, i1, ct, st) in [(0, 1, "cxt", "sxt"), (2, 3, "cyt", "syt")]:
            a, bb = x1T[:, i0], x1T[:, i1]
            t1 = rp.tile([128, T], BF, tag="t1", bufs=2, name="t1")
            t2 = rp.tile([128, T], BF, tag="t2", bufs=2, name="t2")
            t3 = rp.tile([128, T], BF, tag="t3", bufs=2, name="t3")
            t4 = rp.tile([128, T], BF, tag="t4", bufs=2, name="t4")
            nc.vector.tensor_tensor(t1[:], a, rope_t[ct][:], OP.mult)
            nc.vector.tensor_tensor(t2[:], bb, rope_t[st][:], OP.mult)
            nc.gpsimd.tensor_tensor(t3[:], a, rope_t[st][:], OP.mult)
            nc.gpsimd.tensor_tensor(t4[:], bb, rope_t[ct][:], OP.mult)
            nc.vector.tensor_tensor(x1T[:, i0], t1[:], t2[:], OP.subtract)
            nc.vector.tensor_tensor(x1T[:, i1], t3[:], t4[:], OP.add)
        free("p_ropetmp")
    free("p_rope")
    ps_t.release()
    free("p_modtmp")
    free("p_ada")

    # ---------------- helpers ----------------
    def proj_T(name, w_name, src_sb, consumer, forder=None):
        fseq = forder or list(range(HC))
        with nc.named_scope(name):
            for o in range(HC):
                wt = wpool.tile([128, HC, 128], BF, tag="w8", name=f"{name}_w")
                nc.sync.dma_start(wt[:], d[w_name][o])
                for t in range(2):
                    p = P_["proj"].tile([128, 512], F32, tag="proj", name=f"{name}_ps")
                    for i, f in enumerate(fseq):
                        nc.tensor.matmul(p[:], wt[:, f],
                                         src_sb[:, f, t * 512:(t + 1) * 512],
                                         start=(i == 0), stop=(i == HC - 1))
                    consumer(o, t, p)

    def copy_act(dst):
        def c(o, t, p):
            nc.scalar.copy(dst[:, o, t * 512:(t + 1) * 512], p[:])
        return c

    def vnat(w_name, src_sb, dst_v, scope, side="left", forder=None):
        """V natural [toks(128-chunks), 16 heads, 64]."""
        fseq = forder or list(range(HC))
        wv = pool("wv_" + scope, bufs=2, side=side)
        with nc.named_scope(scope):
            for oh in range(2):
                wt = wv.tile([128, HC, 512], BF, tag="wvnat", name=f"{scope}_w")
                nc.sync.dma_start(wt[:], d[w_name][oh])
                for bt in range(2 * BPC):
                    p = P_["proj"].tile([128, 512], F32, tag="proj",
                                     name=f"{scope}_ps")
                    for i, f in enumerate(fseq):
                        nc.tensor.matmul(p[:], src_sb[:, f, bt * 128:(bt + 1) * 128],
                                         wt[:, f], start=(i == 0), stop=(i == HC - 1))
                    nc.scalar.copy(dst_v[:, bt, oh * 8:(oh + 1) * 8, :], p[:])
        free("wv_" + scope)

    def attention(qt_sb, kt_sb, vp_sb, out_sb, scope):
        proj_release()
        at_pool = pool("attnp_" + scope, bufs=1, side="right")
        ps_sc = tc.alloc_tile_pool(name="ps_sc_" + scope, bufs=2, space="PSUM")
        ps_av = tc.alloc_tile_pool(name="ps_av_" + scope, bufs=4, space="PSUM")
        ps_dn = tc.alloc_tile_pool(name="ps_dn_" + scope, bufs=2, space="PSUM")
        with nc.named_scope(scope):
            for b in range(BPC):
                for hc in range(NH // 2):
                    ats = []
                    for ho in range(2):
                        scp = ps_sc.tile([128, 2, S], F32, tag="sc", name="sc_ps")
                        for kc in range(2):
                            nc.tensor.matmul(
                                scp[:, kc],
                                kt_sb[ho * 64:(ho + 1) * 64, hc,
                                      b * S + kc * 128: b * S + (kc + 1) * 128],
                                qt_sb[ho * 64:(ho + 1) * 64, hc, b * S:(b + 1) * S],
                                start=True, stop=True)
                        at = at_pool.tile([128, 2, S], BF, tag="at", bufs=8,
                                          name="at_sb")
                        nc.scalar.activation(at[:], scp[:], AF.Exp,
                                             scale=float(HD) ** -0.5)
                        ats.append(at)
                    for ho in range(2):
                        avd = ps_av.tile([64, S], F32, tag="av", name="av_ps")
                        den = ps_dn.tile([64, S], F32, tag="den", name="den_ps")
                        for kc in range(2):
                            nc.tensor.matmul(avd[:],
                                             vp_sb[:, b * 2 + kc, 2 * hc + ho, :],
                                             ats[ho][:, kc],
                                             start=(kc == 0), stop=(kc == 1))
                        for kc in range(2):
                            nc.tensor.matmul(den[:], ones_sb[:, 0:64],
                                             ats[ho][:, kc],
                                             start=(kc == 0), stop=(kc == 1))
                        inv = at_pool.tile([64, S], F32, tag="inv", bufs=8,
                                           name="inv_sb")
                        nc.vector.reciprocal_approx_fast(inv[:], den[:])
                        nc.vector.tensor_tensor(
                            out_sb[ho * 64:(ho + 1) * 64, hc, b * S:(b + 1) * S],
                            avd[:], inv[:], OP.mult)
        ps_dn.release()
        ps_av.release()
        ps_sc.release()
        free("attnp_" + scope)
        proj_alloc()

    # ---------------- phase A: V1, Q1, K1, attn1, o1 -------------------------
    FR = [4, 5, 6, 7, 0, 1, 2, 3]
    p_vp = pool("p_vp")
    vp = p_vp.tile([128, 2 * BPC, NH, 64], BF)
    vnat("wv1t", x1T, vp, "v1", forder=FR)

    p_qt = pool("p_qt"); qt = p_qt.tile([128, HC, T], BF)
    p_kt = pool("p_kt"); kt = p_kt.tile([128, HC, T], BF)
    proj_T("q1", "wq1t", x1T, copy_act(qt), forder=FR)
    proj_T("k1", "wk1t", x1T, copy_act(kt), forder=FR)
    free("p_x1")

    p_ao = pool("p_ao", side="right")
    attnout = p_ao.tile([128, HC, T], BF)
    attention(qt, kt, vp, attnout, "attn1")
    free("p_kt"); free("p_qt"); free("p_vp")

    def resid_gated(g_tile):
        def c(o, t, p):
            for b2 in range(2):
                b = t * 2 + b2
                sl = slice(t * 512 + b2 * S, t * 512 + (b2 + 1) * S)
                psl = slice(b2 * S, (b2 + 1) * S)
                nc.vector.scalar_tensor_tensor(hsT[:, o, sl], p[:, psl],
                                               g_tile[:, o, b:b + 1],
                                               hsT[:, o, sl], OP.mult, OP.add)
        return c

    proj_T("o1", "wo1t", attnout, resid_gated(modsp[2]))
    free("p_ao")

    # rms2 stats early (hsT now holds h1) — overlaps with eva GEMM
    ms2 = rms_stats("r2")

    # ---------------- phase B: eva, mod j=6..11, k2/v2, rms2, q2, attn2 ------
    p_eva = pool("p_eva")
    evaT = p_eva.tile([128, HC, T], BF)
    p_enc = pool("p_enc", bufs=1)
    wev = pool("p_weva", bufs=3)
    with nc.named_scope("eva"):
        enq = []
        for ch in range(2):
            ench = p_enc.tile([128, EC // 2, T], BF, tag="ench", bufs=2,
                              name="ench")
            nc.sync.dma_start(ench[:], d["enc4"][:, ch * 16:(ch + 1) * 16, :])
            enq.append(ench)
        for o in range(HC):
            wt = wev.tile([128, EC, 128], BF, tag="weva", name="eva_w_t")
            nc.sync.dma_start(wt[:], d["evat"][o])
            for t in range(2):
                p = P_["proj"].tile([128, 512], F32, tag="proj", name="eva_ps")
                for ch in range(2):
                    for f in range(EC // 2):
                        nc.tensor.matmul(p[:], wt[:, ch * 16 + f],
                                         enq[ch][:, f, t * 512:(t + 1) * 512],
                                         start=(ch == 0 and f == 0),
                                         stop=(ch == 1 and f == EC // 2 - 1))
                nc.vector.tensor_scalar(evaT[:, o, t * 512:(t + 1) * 512],
                                        p[:], n_sb["eva_bT"][:, o:o + 1],
                                        None, OP.add)
    free("p_weva")
    free("p_enc")

    p_k2 = pool("p_k2", side="right"); k2t = p_k2.tile([128, HC, T], BF)
    proj_T("k2", "wk2t", evaT, copy_act(k2t))
    p_v2 = pool("p_v2", side="right")
    vp2 = p_v2.tile([128, 2 * BPC, NH, 64], BF)
    vnat("wv2t", evaT, vp2, "v2", side="right")
    free("p_eva")

    p_r2 = pool("p_r2")
    rms2T = p_r2.tile([128, HC, T], BF)

    with nc.named_scope("rms2"):
        rms_apply(ms2, "r2",
                  xn_dst=lambda c, t: rms2T[:, c, t * 512:(t + 1) * 512])

    p_q2 = pool("p_q2", side="right"); q2t = p_q2.tile([128, HC, T], BF)
    proj_T("q2", "wq2t", rms2T, copy_act(q2t))
    free("p_r2")

    p_ao2 = pool("p_ao2", side="right")
    attn2out = p_ao2.tile([128, HC, T], BF)
    attention(q2t, k2t, vp2, attn2out, "attn2")

    def resid_plain(o, t, p):
        nc.vector.tensor_tensor(hsT[:, o, t * 512:(t + 1) * 512],
                                hsT[:, o, t * 512:(t + 1) * 512], p[:], OP.add)

    proj_T("o2", "wo2t", attn2out, resid_plain)
    free("p_ao2"); free("p_q2"); free("p_v2"); free("p_k2")

    # ---------------- phase C: rms3 + MLP ------------------------------------
    with nc.named_scope("scales3"):
        for c in range(HC):
            nc.vector.tensor_scalar(scale3[:, c], modsp[4][:, c], 1.0,
                                    n_sb["n3T"][:, c:c + 1], OP.add, OP.mult)

    ms3 = rms_stats("r3")
    p_y = pool("p_y")
    yT = p_y.tile([128, HC, T], BF)

    def rms3_consumer(c, t, xn):
        for b2 in range(2):
            b = 2 * t + b2
            eng = nc.gpsimd if b2 == 0 else nc.vector
            eng.tensor_scalar(yT[:, c, b * S:(b + 1) * S],
                              xn[:, b2 * S:(b2 + 1) * S],
                              scale3[:, c, b:b + 1],
                              modsp[3][:, c, b:b + 1],
                              OP.mult, OP.add)

    with nc.named_scope("rms3"):
        rms_apply(ms3, "r3", consumer=rms3_consumer)

    p_mlp = pool("p_mlp", side="right")
    mlpT = p_mlp.tile([128, IC, T], BF)
    ps_gu = tc.alloc_tile_pool(name="ps_gu", bufs=4, space="PSUM")
    wmlp = pool("p_wmlp", bufs=8)
    with nc.named_scope("gateup"):
        for o in range(IC):
            wg = wmlp.tile([128, HC, 128], BF, tag="w8", name="gate_w_t")
            nc.sync.dma_start(wg[:], d["gatet"][o])
            wu = wmlp.tile([128, HC, 128], BF, tag="w8", name="up_w_t")
            nc.sync.dma_start(wu[:], d["upt"][o])
            for t in range(2):
                pg = ps_gu.tile([128, 512], F32, tag="gu", name="g_ps")
                for f in range(HC):
                    nc.tensor.matmul(pg[:], wg[:, f],
                                     yT[:, f, t * 512:(t + 1) * 512],
                                     start=(f == 0), stop=(f == HC - 1))
                pu = ps_gu.tile([128, 512], F32, tag="gu", name="u_ps")
                for f in range(HC):
                    nc.tensor.matmul(pu[:], wu[:, f],
                                     yT[:, f, t * 512:(t + 1) * 512],
                                     start=(f == 0), stop=(f == HC - 1))
                gs = sml.tile([128, 512], F32, tag="gsil", bufs=4, name="gsil")
                nc.scalar.activation(gs[:], pg[:], AF.Silu)
                nc.vector.tensor_tensor(mlpT[:, o, t * 512:(t + 1) * 512],
                                        gs[:], pu[:], OP.mult)
    free("p_wmlp")
    free("p_y")

    wdn = pool("p_wdown", bufs=3)
    with nc.named_scope("down"):
        for o in range(HC):
            wt = wdn.tile([128, IC, 128], BF, tag="wdown", name="down_w_t")
            nc.sync.dma_start(wt[:], d["downt"][o])
            for t in range(2):
                p = ps_gu.tile([128, 512], F32, tag="gu", name="d_ps")
                for f in range(IC):
                    nc.tensor.matmul(p[:], wt[:, f],
                                     mlpT[:, f, t * 512:(t + 1) * 512],
                                     start=(f == 0), stop=(f == IC - 1))
                fin = sml.tile([128, 512], F32, tag="fin", bufs=4, name="fin")
                for b2 in range(2):
                    b = t * 2 + b2
                    psl = slice(b2 * S, (b2 + 1) * S)
                    sl = slice(t * 512 + b2 * S, t * 512 + (b2 + 1) * S)
                    nc.vector.scalar_tensor_tensor(fin[:, psl], p[:, psl],
                                                   modsp[5][:, o, b:b + 1],
                                                   hsT[:, o, sl], OP.mult, OP.add)
                nc.sync.dma_start(outT_d[:, o, t * 512:(t + 1) * 512], fin[:])
    free("p_wdown")
    free("p_mlp")

    for nm in reversed(list(open_pools)):
        free(nm)
    ps_gu.release()
    proj_release()
    tc_cm.__exit__(None, None, None)
    nc.compile()
    return nc


_CACHE = {}


def _get_program():
    if "nc" not in _CACHE:
        _CACHE["nc"] = build_program()
    return _CACHE["nc"]


def kernel(hidden_states, encoder_hidden_states, timestep_emb,
           wq1, wk1, wv1, wo1, wq2, wk2, wv2, wo2,
           eva_w, eva_b, ada_w, ada_b, gate_w, up_w, down_w, n1, n2, n3,
           _trace=False):
    from ml_dtypes import bfloat16
    nc = _get_program()
    f32 = lambda a: np.ascontiguousarray(np.asarray(a), dtype=np.float32)
    bf = lambda a: np.ascontiguousarray(np.asarray(a, dtype=np.float32)
                                        .astype(bfloat16))

    cxt, sxt, cyt, syt = _rope_tables()
    colchunks = lambda v, n: np.ascontiguousarray(
        np.asarray(v, np.float32).reshape(n, 128).T)

    # pre-tiled weight layouts: per-o tiles, contiguous per-partition lines
    def qkvo_t(w, cv=None):  # [H, H] -> [8, 128, 8, 128]; tile[o][p,f,m] = w[f*128+p, o*128+m]
        cv = cv or bf
        return cv(np.asarray(w, np.float32).reshape(HC, 128, HC, 128)
                  .transpose(2, 1, 0, 3))

    def vnat_t(w, cv=None):  # [H, H] -> [2, 128, 8, 512]
        cv = cv or bf
        return cv(np.asarray(w, np.float32).reshape(HC, 128, 2, 512)
                  .transpose(2, 1, 0, 3))

    def kc_t(w, KC, OC, cv=None):  # [K, M] -> [OC, 128, KC, 128]
        cv = cv or bf
        return cv(np.asarray(w, np.float32).reshape(KC, 128, OC, 128)
                  .transpose(2, 1, 0, 3))

    shared = dict(
        wq1t=qkvo_t(wq1), wk1t=qkvo_t(wk1), wo1t=qkvo_t(wo1),
        wq2t=qkvo_t(np.asarray(n2, np.float32)[:, None]
                    * np.asarray(wq2, np.float32)),
        wk2t=qkvo_t(wk2), wo2t=qkvo_t(wo2),
        wv1t=vnat_t(wv1), wv2t=vnat_t(wv2),
        evat=kc_t(eva_w, EC, HC),
        adat=bf(np.asarray(ada_w, np.float32).reshape(HC, 128, 12, 512)
                .transpose(2, 1, 0, 3)),
        adab=bf(np.asarray(ada_b, np.float32).reshape(1, 6 * H)),
        gatet=kc_t(gate_w, HC, IC), upt=kc_t(up_w, HC, IC),
        downt=kc_t(down_w, IC, HC),
        eva_bT=colchunks(eva_b, HC),
        n1T=colchunks(n1, HC), n3T=colchunks(n3, HC),
        cxt=bf(cxt), sxt=bf(sxt), cyt=bf(cyt), syt=bf(syt),
        ones_bf=bf(np.ones((128, 128), np.float32)),
        eye4=np.eye(4, dtype=np.float32),
        epsc=np.full((128, 1), EPS, np.float32),
    )
    hs = f32(hidden_states)
    enc = np.asarray(encoder_hidden_states)
    temb = f32(timestep_emb)

    in_maps = []
    for c in range(NC_):
        sl = slice(c * BPC, (c + 1) * BPC)
        m = dict(shared)
        m["hsT4"] = np.ascontiguousarray(
            hs[sl].transpose(2, 0, 1).reshape(HC, 128, T).transpose(1, 0, 2))
        m["enc4"] = bf(np.asarray(enc[sl], np.float32).transpose(2, 0, 1)
                       .reshape(EC, 128, T).transpose(1, 0, 2))
        m["stembT"] = np.ascontiguousarray(
            temb[sl].reshape(BPC, HC, 128).transpose(2, 1, 0))
        in_maps.append(m)

    res = run_bass_kernel_spmd(nc, in_maps, core_ids=list(range(NC_)),
                               trace=_trace)
    out = np.empty((B, S, H), np.float32)
    for c in range(NC_):
        o4 = res.results[c]["outT"]  # [128, 8, 1024]
        oT = np.ascontiguousarray(o4.transpose(1, 0, 2)).reshape(H, T)
        out[c * BPC:(c + 1) * BPC] = np.ascontiguousarray(oT.T).reshape(BPC, S, H)
    if _trace:
        kernel.last_results = res
    return out
